# revision 17
# baseline (speedup 1.0000x reference)
"""Trainium2 Bass kernel for nn_EntropyMaskGate.

Pipeline per core (2 images, batch-sharded over 8 cores):
  conv1 (1x1, 256->64) -> gelu -> grouped 3x3 conv (SAME, 8 groups) -> gelu
  -> conv3 (1x1, 64->256) + bias  = entropy_scores (f32, DMA'd full-res)
  2x2 block sums of scores (two big DVE adds) -> per-(b,c)-row 256th-smallest
  threshold (bf16 bisection in sigma-normalized bracket + exact f32 top-8
  finisher) -> Sign-mask written as fp8 block mask; host upsamples 2x2.

Forward-pass note: the reference's STE expression (sg(hard) - sg(soft) + soft)
is exactly `hard` in fp32 round-to-nearest (soft is clipped to [0,1]), so the
mask output is the upsampled hard mask; the soft path is numerically dead.

Precision notes (measured on this hardware):
  - PE fp32 matmul is fp32-exact (rel l2 ~1e-7 vs fp64); fp32r is tf32-like
    (1.4e-4) so it is NOT used: impp = pool(conv3) feeds the mask and needs
    exact ordering.
  - Bisection counts run on bf16 copies (centered per row); only the bracket
    comes from them, the threshold itself is recovered from exact f32 values
    via top-8 (max8) with a 0.005*sigma guard band on the bracket low edge.
  - ACT Sign(thr - impp) -> {-1,0,+1} exact in fp8_e4m3; host keeps >= 0
    (sign==0 is the threshold element itself, which reference keeps).
"""

import numpy as np

import concourse.bass as bass
import concourse.mybir as mybir
from concourse import bacc, bass_utils
from concourse.tile import TileContext

F32 = mybir.dt.float32
BF16 = mybir.dt.bfloat16
FP8 = mybir.dt.float8e4
I32 = mybir.dt.int32
U8 = mybir.dt.uint8
AF = mybir.ActivationFunctionType
OP = mybir.AluOpType

B, C, H, W = 16, 256, 64, 64
MID, GROUPS = 64, 8
N_CORES = 8
IMGS = B // N_CORES          # 2 images per core
HW = H * W                   # 4096
NBLK = 1024                  # 32*32 blocks per (b,c) row
KEEP = 256                   # blocks kept per row
T_BISECT = 12                # bisection iterations
BR_LO = -1.05                # bracket in centered/sigma units around the
BR_HI = -0.35                # 25th-percentile threshold (z ~ -0.674)
K_SEARCH = 253               # bf16 bisection target: 3 ranks below KEEP
                             # absorbs bf16-vs-f32 count slack (+-~2)
                             # while keeping the finisher rank in [1,8]
BIGNEG = -(2.0 ** 96)

TRACE = False
LAST_RESULTS = None
import os as _os
DBG_TBI = int(_os.environ.get("KM_TBI", str(T_BISECT)))
DBG_SKIPFIN = int(_os.environ.get("KM_SKIPFIN", "0"))
DBG_SKIPSEARCH = int(_os.environ.get("KM_SKIPSEARCH", "0"))
DBG_NOMASK = int(_os.environ.get("KM_NOMASK", "0"))
DBG_NOSCORES = int(_os.environ.get("KM_NOSCORES", "0"))
DBG_NOIMPP = int(_os.environ.get("KM_NOIMPP", "0"))


def build_nc(repeat=1):
    nc = bacc.Bacc("TRN2", target_bir_lowering=False, debug=False,
                   num_devices=N_CORES)

    feats_d = nc.dram_tensor("features", [IMGS, C, HW], F32, kind="ExternalInput").ap()
    w1t_d = nc.dram_tensor("w1t", [128, 2, 128], F32, kind="ExternalInput").ap()
    w2p_d = nc.dram_tensor("w2p", [128, 3, MID], F32, kind="ExternalInput").ap()
    w2s_d = nc.dram_tensor("w2s", [MID, 3, MID], F32, kind="ExternalInput").ap()
    w3t_d = nc.dram_tensor("w3t", [MID, C], F32, kind="ExternalInput").ap()
    b1_d = nc.dram_tensor("b1", [128, 1], F32, kind="ExternalInput").ap()
    b2_d = nc.dram_tensor("b2", [MID, 1], F32, kind="ExternalInput").ap()
    b3s_d = nc.dram_tensor("b3s", [128, 2], F32, kind="ExternalInput").ap()
    scores_d = nc.dram_tensor("scores", [IMGS, C, HW], F32, kind="ExternalOutput").ap()
    mask_d = nc.dram_tensor("mask", [IMGS, C, NBLK], FP8, kind="ExternalOutput").ap()

    with TileContext(nc) as tc:
        for _rep in range(repeat):
            _build(nc, tc, feats_d, w1t_d, w2p_d, w2s_d, w3t_d, b1_d, b2_d,
                   b3s_d, scores_d, mask_d)
    nc.compile()
    return nc


def _build(nc, tc, feats_d, w1t_d, w2p_d, w2s_d, w3t_d, b1_d, b2_d, b3s_d,
           scores_d, mask_d):
    cpool = tc.alloc_tile_pool(name="consts", bufs=1)
    xpool = tc.alloc_tile_pool(name="x", bufs=1)
    h1pool = tc.alloc_tile_pool(name="h1", bufs=2)
    h2pool = tc.alloc_tile_pool(name="h2", bufs=6)
    spool = tc.alloc_tile_pool(name="s", bufs=1)
    cppool = tc.alloc_tile_pool(name="cp", bufs=1)
    ipool = tc.alloc_tile_pool(name="impp", bufs=2)
    bpool = tc.alloc_tile_pool(name="impb", bufs=2)
    npool = tc.alloc_tile_pool(name="neg", bufs=1)
    scrpool = tc.alloc_tile_pool(name="scr", bufs=2)
    mpool = tc.alloc_tile_pool(name="maskb", bufs=2)
    stpool = tc.alloc_tile_pool(name="stats", bufs=1)
    ps1 = tc.alloc_tile_pool(name="ps1", bufs=2, space="PSUM")
    ps2 = tc.alloc_tile_pool(name="ps2", bufs=4, space="PSUM")
    ps3 = tc.alloc_tile_pool(name="ps3", bufs=2, space="PSUM")

    # ---- constants ----
    w1_sb = cpool.tile([128, 2, 128], F32, name="w1", tag="w1")
    nc.sync.dma_start(out=w1_sb[:], in_=w1t_d[:])
    w2p_sb = cpool.tile([128, 3, MID], F32, name="w2p", tag="w2p")
    nc.sync.dma_start(out=w2p_sb[:], in_=w2p_d[:])
    w2s_sb = cpool.tile([MID, 3, MID], F32, name="w2s", tag="w2s")
    nc.sync.dma_start(out=w2s_sb[:], in_=w2s_d[:])
    w3_sb = cpool.tile([MID, C], F32, name="w3", tag="w3")
    nc.sync.dma_start(out=w3_sb[:], in_=w3t_d[:])
    b1_sb = cpool.tile([128, 1], F32, name="b1", tag="b1")
    nc.sync.dma_start(out=b1_sb[:], in_=b1_d[:])
    b2_sb = cpool.tile([MID, 1], F32, name="b2", tag="b2")
    nc.sync.dma_start(out=b2_sb[:], in_=b2_d[:])
    b3s_sb = cpool.tile([128, 2], F32, name="b3s", tag="b3s")
    nc.sync.dma_start(out=b3s_sb[:], in_=b3s_d[:])

    iota_i = cpool.tile([128, 8], I32, name="iotai", tag="iotai")
    nc.gpsimd.iota(iota_i[:], pattern=[[1, 8]], base=0, channel_multiplier=0)
    iotaneg = cpool.tile([128, 8], F32, name="iotan", tag="iotan")
    nc.vector.tensor_copy(iotaneg[:], iota_i[:])
    nc.vector.tensor_scalar(iotaneg[:], iotaneg[:], -1.0, None, op0=OP.mult)
    negbig = cpool.tile([128, NBLK], F32, name="negbig", tag="negbig")
    nc.vector.memset(negbig[:], BIGNEG)

    # ---- per-image threshold search + mask from pooled scores impp ----
    def _threshold_and_mask(img, impp, maskb):
        def st_tile(tag, cols=2):
            return stpool.tile([128, cols], F32, name=f"{tag}{img}",
                               tag=f"{tag}{img}")
        agg, mu, var, sig, negmu = (st_tile(t) for t in
                                    ["agg", "mu", "var", "sig", "negmu"])
        lo, hi, mid, cnt, lof = (st_tile(t) for t in
                                 ["lo", "hi", "mid", "cnt", "lof"])
        cntlos, jneg, thrneg, thr, tmp = (st_tile(t) for t in
                                          ["cntlos", "jneg", "thrneg", "thr",
                                           "tmp"])
        sgek = stpool.tile([128, 2], U8, name=f"sgek{img}", tag=f"sgek{img}")
        sltk = stpool.tile([128, 2], U8, name=f"sltk{img}", tag=f"sltk{img}")
        agg2 = stpool.tile([128, 2, 2], F32, name=f"agg2{img}", tag=f"agg2{img}")

        # per-row mean/var via bn_stats (input viewed as 2 groups of 512)
        for rt in range(2):
            bs6 = stpool.tile([128, 2, 6], F32, name=f"bs{img}{rt}",
                              tag=f"bs{img}{rt}")
            for g in range(2):
                nc.vector.bn_stats(bs6[:, g, :], impp[:, rt, g * 512:(g + 1) * 512])
            nc.vector.bn_aggr(agg2[:, rt, :], bs6[:])
        nc.vector.tensor_copy(mu[:], agg2[:, :, 0])
        nc.vector.tensor_copy(var[:], agg2[:, :, 1])
        nc.scalar.activation(sig[:], var[:], AF.Sqrt)
        nc.vector.tensor_scalar(negmu[:], mu[:], -1.0, None, op0=OP.mult)

        # centered bf16 copy for the bisection counts
        impb = bpool.tile([128, 2, NBLK], BF16, name=f"impb{img}",
                          tag="impb")
        for rt in range(2):
            nc.scalar.activation(impb[:, rt, :], impp[:, rt, :], AF.Identity,
                                 bias=negmu[:, rt:rt + 1])

        # bracket (centered coords): [BR_LO, BR_HI] * sigma
        nc.vector.tensor_scalar(lo[:], sig[:], BR_LO, None, op0=OP.mult)
        nc.vector.tensor_scalar(hi[:], sig[:], BR_HI, None, op0=OP.mult)
        nc.vector.tensor_add(mid[:], lo[:], hi[:])
        nc.vector.tensor_scalar(mid[:], mid[:], 0.5, None, op0=OP.mult)

        for it in range(0 if DBG_SKIPSEARCH else DBG_TBI):
            for rt in range(2):
                scr = scrpool.tile([128, NBLK], BF16, name="scr",
                                   tag=f"scr{rt}")
                nc.vector.tensor_scalar(scr[:], impb[:, rt, :],
                                        mid[:, rt:rt + 1], None, op0=OP.is_le,
                                        op1=OP.add,
                                        accum_out=cnt[:, rt:rt + 1])
            nc.vector.tensor_scalar(sgek[:], cnt[:], float(K_SEARCH), None,
                                    op0=OP.is_ge)
            nc.vector.copy_predicated(hi[:], sgek[:], mid[:])
            nc.vector.tensor_scalar(sltk[:], cnt[:], float(K_SEARCH), None,
                                    op0=OP.is_lt)
            nc.vector.copy_predicated(lo[:], sltk[:], mid[:])
            nc.vector.tensor_add(mid[:], lo[:], hi[:])
            nc.vector.tensor_scalar(mid[:], mid[:], 0.5, None, op0=OP.mult)

        # absolute low edge of the bracket
        nc.vector.tensor_add(lof[:], lo[:], mu[:])

        # exact f32 finisher: v_k = (256 - cnt_le(lof))-th smallest > lof
        negimp = npool.tile([128, 2, NBLK], F32, name=f"negimp{img}",
                            tag="negimp")
        nc.scalar.activation(negimp[:], impp[:], AF.Copy, scale=-1.0)
        for rt in range(2 if not DBG_SKIPFIN else 0):
            mle = scrpool.tile([128, NBLK], U8, name="mle", tag=f"mleu{rt}")
            nc.vector.tensor_scalar(mle[:], impp[:, rt, :],
                                    lof[:, rt:rt + 1], None, op0=OP.is_le,
                                    op1=OP.add,
                                    accum_out=cntlos[:, rt:rt + 1])
            nc.vector.copy_predicated(negimp[:, rt, :], mle[:], negbig[:])
            top8 = stpool.tile([128, 8], F32, name=f"top8_{img}{rt}",
                               tag=f"top8_{img}{rt}")
            nc.vector.max(out=top8[:], in_=negimp[:, rt, :])
            nc.vector.tensor_scalar(jneg[:, rt:rt + 1], cntlos[:, rt:rt + 1],
                                    -255.0, None, op0=OP.add)
            eq8 = stpool.tile([128, 8], F32, name=f"eq8_{img}{rt}",
                              tag=f"eq8_{img}{rt}")
            nc.vector.tensor_scalar(eq8[:], iotaneg[:], jneg[:, rt:rt + 1],
                                    None, op0=OP.is_equal)
            scr8 = stpool.tile([128, 8], F32, name=f"scr8_{img}{rt}",
                               tag=f"scr8_{img}{rt}")
            nc.vector.tensor_mul(scr8[:], top8[:], eq8[:])
            nc.vector.tensor_reduce(thrneg[:, rt:rt + 1], scr8[:],
                                    axis=mybir.AxisListType.X, op=OP.add)
        if DBG_SKIPFIN:
            nc.vector.memset(thr[:], 0.0)
        else:
            nc.vector.tensor_scalar(thr[:], thrneg[:], -1.0, None,
                                    op0=OP.mult)
            # cnt_le(lof) >= 256: the bracket edge itself is a valid
            # threshold (keeps cnt_le(lof) blocks, ==256 in practice)
            mneg = stpool.tile([128, 2], U8, name=f"mneg{img}",
                               tag=f"mneg{img}")
            nc.vector.tensor_scalar(mneg[:], cntlos[:], 256.0, None,
                                    op0=OP.is_ge)
            nc.vector.copy_predicated(thr[:], mneg[:], lof[:])

        # fp8 block mask: Sign(thr - impp) in {-1,0,+1}; host keeps >= 0
        for rt in range(2):
            nc.scalar.activation(maskb[:, rt, :], impp[:, rt, :], AF.Sign,
                                 bias=thr[:, rt:rt + 1], scale=-1.0)

    # ---- per-image conv stack ----
    for img in range(IMGS):
        x0 = xpool.tile([128, HW], F32, name="x0", tag="x0")
        x1 = xpool.tile([128, HW], F32, name="x1", tag="x1")
        for dc in range(4):
            cs = dc * (HW // 4)
            nc.sync.dma_start(out=x0[:, cs:cs + HW // 4],
                              in_=feats_d[img, 0:128, cs:cs + HW // 4])
            nc.sync.dma_start(out=x1[:, cs:cs + HW // 4],
                              in_=feats_d[img, 128:256, cs:cs + HW // 4])

        # h1 with one-pixel zero halo, [66 x 66]. Partitions 0-63 hold h1;
        # partitions 64-127 hold a copy shifted one column left
        # (u[r, c] = h1pad[r, c+1]) so a single full-height AP feeds the
        # (dy,0)+(dy,1) tap pair of conv2 as one K=128 matmul.
        h1t = h1pool.tile([128, 66 * 66], F32, name="h1", tag="h1")
        h1v = h1t[0:64, :].rearrange("p (r c) -> p r c", c=66)
        h1u = h1t[64:128, :].rearrange("p (r c) -> p r c", c=66)
        nc.vector.memset(h1v[:, 0:1, :], 0.0)
        nc.vector.memset(h1v[:, 65:66, :], 0.0)
        nc.vector.memset(h1v[:, 1:65, 0:1], 0.0)
        nc.vector.memset(h1v[:, 1:65, 65:66], 0.0)
        nc.vector.memset(h1u[:, 0:1, :], 0.0)
        nc.vector.memset(h1u[:, 65:66, :], 0.0)
        nc.vector.memset(h1u[:, 1:65, 64:65], 0.0)

        # conv1 (1x1, M duplicated) + gelu into both h1 copies.
        # Chunk pairs interleaved so the K-accumulation latency of one psum
        # chain hides under the other.
        for cp2 in range(4):
            pts = []
            for half in range(2):
                ci = 2 * cp2 + half
                pts.append(ps1.tile([128, 512], F32, name="ps1", tag="ps1"))
            for k in range(2):
                for half in range(2):
                    ci = 2 * cp2 + half
                    cs = ci * 512
                    xk = x0 if k == 0 else x1
                    nc.tensor.matmul(pts[half][:], w1_sb[:, k, :],
                                     xk[:, cs:cs + 512],
                                     start=(k == 0), stop=(k == 1))
            for half in range(2):
                ci = 2 * cp2 + half
                pt = pts[half]
                nc.scalar.activation(h1v[:, 1 + 8 * ci:9 + 8 * ci, 1:65],
                                     pt[0:64, :].rearrange("p (r c) -> p r c", c=64),
                                     AF.Gelu, bias=b1_sb[0:64, 0:1])
                nc.scalar.activation(h1u[:, 1 + 8 * ci:9 + 8 * ci, 0:64],
                                     pt[64:128, :].rearrange("p (r c) -> p r c", c=64),
                                     AF.Gelu, bias=b1_sb[64:128, 0:1])

        # conv2 (grouped 3x3 SAME) + gelu -> h2 in 16-row tiles.
        # 4 accumulation chains interleaved (2 c2-tiles x 2 halves) so the
        # psum-accumulate latency overlaps across chains.
        h1f = h1t[:].rearrange("p (r c) -> p r c", c=66)
        h2_tiles = []
        for c2q in range(2):
            quad = []           # (pt, h2t, half, ci)
            for dc in range(2):
                c2 = 2 * c2q + dc
                h2t = h2pool.tile([MID, 1024], F32, name="h2", tag="h2")
                h2_tiles.append(h2t)
                for half in range(2):
                    ci = 2 * c2 + half
                    pt = ps2.tile([MID, 512], F32, name="ps2",
                                  tag="ps2")
                    quad.append((pt, h2t, half, ci))
            for t in range(6):
                for pt, h2t, half, ci in quad:
                    r0 = 8 * ci
                    if t < 3:      # pair (dy,0)+(dy,1), K=128 over both copies
                        dy = t
                        lhsT = w2p_sb[:, dy, :]
                        rhs = h1f[:, r0 + dy:r0 + dy + 8, 0:64]
                    else:          # single (dy,2), K=64 lower copy
                        dy = t - 3
                        lhsT = w2s_sb[:, dy, :]
                        rhs = h1v[:, r0 + dy:r0 + dy + 8, 2:66]
                    nc.tensor.matmul(
                        pt[:].rearrange("p (r c) -> p r c", c=64),
                        lhsT, rhs, start=(t == 0), stop=(t == 5))
            for pt, h2t, half, ci in quad:
                nc.scalar.activation(h2t[:, half * 512:(half + 1) * 512], pt[:],
                                     AF.Gelu, bias=b2_sb[:, 0:1])

        # conv3 (1x1, 64->256) + bias -> full-res scores tile, one DMA
        sfull = spool.tile([128, 2, HW], F32, name="sfull", tag="sfull")
        for ci in range(8):
            h2t = h2_tiles[ci // 2]
            for mt in range(2):
                pt = ps3.tile([128, 512], F32, name="ps3", tag="ps3")
                nc.tensor.matmul(
                    pt[:], w3_sb[:, mt * 128:(mt + 1) * 128],
                    h2t[:, (ci % 2) * 512:(ci % 2 + 1) * 512],
                    start=True, stop=True)
                if mt == 0:
                    nc.scalar.activation(
                        sfull[:, mt, ci * 512:(ci + 1) * 512], pt[:],
                        AF.Identity, bias=b3s_sb[:, mt:mt + 1])
                else:
                    nc.vector.tensor_scalar(
                        sfull[:, mt, ci * 512:(ci + 1) * 512], pt[:],
                        b3s_sb[:, mt:mt + 1], None, op0=OP.add)
        if not DBG_NOSCORES:
            nc.sync.dma_start(
                out=scores_d[img].rearrange("(t c) w -> c t w", t=2),
                in_=sfull[:])

        # impp = 2x2 block sums of sfull (two big DVE adds)
        impp = ipool.tile([128, 2, NBLK], F32, name="impp", tag="impp")
        if DBG_NOIMPP:
            nc.vector.memset(impp[:], 0.0)
        else:
            sv = sfull[:].rearrange("p t (r c w) -> p t r c w", c=32, w=2)
            cp = cppool.tile([128, 2, 64, 32], F32, name="cp", tag="cp")
            nc.vector.tensor_tensor(cp[:], sv[:, :, :, :, 0], sv[:, :, :, :, 1],
                                    op=OP.add)
            cpv = cp[:].rearrange("p t (r w) c -> p t r w c", w=2)
            nc.vector.tensor_tensor(
                impp[:].rearrange("p t (r c) -> p t r c", c=32),
                cpv[:, :, :, 0, :], cpv[:, :, :, 1, :], op=OP.add)

        maskb = mpool.tile([128, 2, NBLK], FP8, name="maskb", tag="maskb")
        _threshold_and_mask(img, impp, maskb)
        if not DBG_NOMASK:
            nc.sync.dma_start(
                out=mask_d[img].rearrange("(t c) w -> c t w", t=2),
                in_=maskb[:])

    for _p in (ps3, ps2, ps1, stpool, mpool, scrpool, npool, bpool, ipool,
               cppool, spool, h2pool, h1pool, xpool, cpool):
        _p.release()


def _prep_weights(w1, b1, w2, b2, w3, b3):
    w1t = np.ascontiguousarray(
        w1[:, :, 0, 0].T.reshape(2, 128, MID).transpose(1, 0, 2)).astype(np.float32)
    w1d = np.concatenate([w1t, w1t], axis=2)      # [128, 2, 128]: M duplicated
    w2t = np.zeros((MID, 9, MID), np.float32)
    for m in range(MID):
        g = m // 8
        for dy in range(3):
            for dx in range(3):
                w2t[g * 8:(g + 1) * 8, 3 * dy + dx, m] = w2[m, :, dy, dx]
    # K=128 pairs: rows 0-63 tap (dy,0), rows 64-127 tap (dy,1); the upper
    # h1 copy is column-shifted -1 so one AP offset feeds both taps.
    w2p = np.stack([np.concatenate([w2t[:, 3 * dy + 0, :],
                                    w2t[:, 3 * dy + 1, :]], axis=0)
                    for dy in range(3)], axis=1)  # [128, 3, 64]
    w2s = np.ascontiguousarray(w2t[:, [2, 5, 8], :])  # [64, 3, 64] taps (dy,2)
    w3t = np.ascontiguousarray(w3[:, :, 0, 0].T).astype(np.float32)
    b3s = np.ascontiguousarray(b3.reshape(2, 128).T).astype(np.float32)
    b1d = np.concatenate([b1.reshape(MID, 1)] * 2, 0).astype(np.float32)
    return dict(w1t=w1d, w2p=w2p, w2s=w2s, w3t=w3t,
                b1=b1d,
                b2=b2.reshape(MID, 1).astype(np.float32),
                b3s=b3s)


_nc_cache = None


def kernel(features, w1, b1, w2, b2, w3, b3, enabled):
    global _nc_cache, LAST_RESULTS
    features = np.asarray(features, np.float32)
    if not int(np.asarray(enabled)):
        return (np.ones((B, C, H, W), np.float32),
                np.zeros((B, C, H, W), np.float32))
    if _nc_cache is None:
        _nc_cache = build_nc()
    nc = _nc_cache
    wmap = _prep_weights(np.asarray(w1), np.asarray(b1), np.asarray(w2),
                         np.asarray(b2), np.asarray(w3), np.asarray(b3))
    fr = features.reshape(B, C, HW)
    in_maps = [dict(features=fr[c * IMGS:(c + 1) * IMGS], **wmap)
               for c in range(N_CORES)]
    res = bass_utils.run_bass_kernel_spmd(nc, in_maps, list(range(N_CORES)),
                                          trace=TRACE)
    LAST_RESULTS = res
    maskb = np.concatenate(
        [np.asarray(res.results[c]["mask"]).astype(np.float32)
         for c in range(N_CORES)], 0)          # [B, C, 1024] in {-1, 0, +1}
    scores = np.concatenate([res.results[c]["scores"] for c in range(N_CORES)], 0)
    blocks = (maskb >= 0).reshape(B, C, 32, 32)
    full = np.broadcast_to(blocks[:, :, :, None, :, None],
                           (B, C, 32, 2, 32, 2)).reshape(B, C, H, W)
    return (full.astype(np.float32),
            scores.reshape(B, C, H, W).astype(np.float32))


if __name__ == "__main__":
    nc = build_nc()
    print("build + compile OK")


# revision 20
# speedup vs baseline: 1.3069x; 1.3069x over previous
"""Trainium2 Bass kernel for nn_EntropyMaskGate.

Pipeline per core (2 images, batch-sharded over 8 cores):
  conv1 (1x1, 256->64) -> gelu -> grouped 3x3 conv (SAME, 8 groups) -> gelu
  -> conv3 (1x1, 64->256) + bias  = entropy_scores (f32, DMA'd full-res)
  2x2 block sums of scores (two big DVE adds) -> per-(b,c)-row 256th-smallest
  threshold (bf16 bisection in sigma-normalized bracket + exact f32 top-8
  finisher) -> Sign-mask written as fp8 block mask; host upsamples 2x2.

Forward-pass note: the reference's STE expression (sg(hard) - sg(soft) + soft)
is exactly `hard` in fp32 round-to-nearest (soft is clipped to [0,1]), so the
mask output is the upsampled hard mask; the soft path is numerically dead.

Precision notes (measured on this hardware):
  - PE fp32 matmul is fp32-exact (rel l2 ~1e-7 vs fp64); fp32r is tf32-like
    (1.4e-4) so it is NOT used: impp = pool(conv3) feeds the mask and needs
    exact ordering.
  - Bisection counts run on bf16 copies (centered per row); only the bracket
    comes from them, the threshold itself is recovered from exact f32 values
    via top-8 (max8) with a 0.005*sigma guard band on the bracket low edge.
  - ACT Sign(thr - impp) -> {-1,0,+1} exact in fp8_e4m3; host keeps >= 0
    (sign==0 is the threshold element itself, which reference keeps).
"""

import numpy as np

import concourse.bass as bass
import concourse.mybir as mybir
from concourse import bacc, bass_utils
from concourse.tile import TileContext

F32 = mybir.dt.float32
BF16 = mybir.dt.bfloat16
FP8 = mybir.dt.float8e4
I32 = mybir.dt.int32
U8 = mybir.dt.uint8
AF = mybir.ActivationFunctionType
OP = mybir.AluOpType

B, C, H, W = 16, 256, 64, 64
MID, GROUPS = 64, 8
N_CORES = 8
IMGS = B // N_CORES          # 2 images per core
HW = H * W                   # 4096
NBLK = 1024                  # 32*32 blocks per (b,c) row
KEEP = 256                   # blocks kept per row
T_BISECT = 12                # bisection iterations
BR_LO = -1.05                # bracket in centered/sigma units around the
BR_HI = -0.35                # 25th-percentile threshold (z ~ -0.674)
K_SEARCH = 253               # bf16 bisection target: 3 ranks below KEEP
                             # absorbs bf16-vs-f32 count slack (+-~2)
                             # while keeping the finisher rank in [1,8]
BIGNEG = -(2.0 ** 96)

TRACE = False
LAST_RESULTS = None
import os as _os
DBG_TBI = int(_os.environ.get("KM_TBI", str(T_BISECT)))
DBG_SKIPFIN = int(_os.environ.get("KM_SKIPFIN", "0"))
DBG_SKIPSEARCH = int(_os.environ.get("KM_SKIPSEARCH", "0"))
DBG_NOMASK = int(_os.environ.get("KM_NOMASK", "0"))
DBG_NOSCORES = int(_os.environ.get("KM_NOSCORES", "0"))
DBG_NOIMPP = int(_os.environ.get("KM_NOIMPP", "0"))
DBG_NOCONV3 = int(_os.environ.get("KM_NOCONV3", "0"))


def build_nc(repeat=1):
    nc = bacc.Bacc("TRN2", target_bir_lowering=False, debug=False,
                   num_devices=N_CORES)

    feats_d = nc.dram_tensor("features", [IMGS, C, HW], F32, kind="ExternalInput").ap()
    w1t_d = nc.dram_tensor("w1t", [128, 2, 128], F32, kind="ExternalInput").ap()
    w2p_d = nc.dram_tensor("w2p", [128, 3, MID], F32, kind="ExternalInput").ap()
    w2s_d = nc.dram_tensor("w2s", [MID, 3, MID], F32, kind="ExternalInput").ap()
    w3t_d = nc.dram_tensor("w3t", [MID, C], F32, kind="ExternalInput").ap()
    b1_d = nc.dram_tensor("b1", [128, 1], F32, kind="ExternalInput").ap()
    b2_d = nc.dram_tensor("b2", [MID, 1], F32, kind="ExternalInput").ap()
    b3s_d = nc.dram_tensor("b3s", [128, 2], F32, kind="ExternalInput").ap()
    scores_d = nc.dram_tensor("scores", [IMGS, C, HW], F32, kind="ExternalOutput").ap()
    mask_d = nc.dram_tensor("mask", [IMGS, C, NBLK], FP8, kind="ExternalOutput").ap()

    with TileContext(nc) as tc:
        for _rep in range(repeat):
            _build(nc, tc, feats_d, w1t_d, w2p_d, w2s_d, w3t_d, b1_d, b2_d,
                   b3s_d, scores_d, mask_d)
    nc.compile()
    return nc


def _build(nc, tc, feats_d, w1t_d, w2p_d, w2s_d, w3t_d, b1_d, b2_d, b3s_d,
           scores_d, mask_d):
    cpool = tc.alloc_tile_pool(name="consts", bufs=1)
    xpool = tc.alloc_tile_pool(name="x", bufs=1)
    h1pool = tc.alloc_tile_pool(name="h1", bufs=2)
    h2pool = tc.alloc_tile_pool(name="h2", bufs=6)
    spool = tc.alloc_tile_pool(name="s", bufs=1)
    cppool = tc.alloc_tile_pool(name="cp", bufs=1)
    ipool = tc.alloc_tile_pool(name="impp", bufs=2)
    bpool = tc.alloc_tile_pool(name="impb", bufs=1)  # 2 tags, one per img
    npool = tc.alloc_tile_pool(name="neg", bufs=1)  # 2 tags, one per img
    scrpool = tc.alloc_tile_pool(name="scr", bufs=1)
    mpool = tc.alloc_tile_pool(name="maskb", bufs=2)
    stpool = tc.alloc_tile_pool(name="stats", bufs=1)
    psp = tc.alloc_tile_pool(name="psp", bufs=8, space="PSUM")

    # ---- constants ----
    w1_sb = cpool.tile([128, 2, 128], F32, name="w1", tag="w1")
    nc.sync.dma_start(out=w1_sb[:], in_=w1t_d[:])
    w2p_sb = cpool.tile([128, 3, MID], F32, name="w2p", tag="w2p")
    nc.sync.dma_start(out=w2p_sb[:], in_=w2p_d[:])
    w2s_sb = cpool.tile([MID, 3, MID], F32, name="w2s", tag="w2s")
    nc.sync.dma_start(out=w2s_sb[:], in_=w2s_d[:])
    w3_sb = cpool.tile([MID, C], F32, name="w3", tag="w3")
    nc.sync.dma_start(out=w3_sb[:], in_=w3t_d[:])
    b1_sb = cpool.tile([128, 1], F32, name="b1", tag="b1")
    nc.sync.dma_start(out=b1_sb[:], in_=b1_d[:])
    b2_sb = cpool.tile([MID, 1], F32, name="b2", tag="b2")
    nc.sync.dma_start(out=b2_sb[:], in_=b2_d[:])
    b3s_sb = cpool.tile([128, 2], F32, name="b3s", tag="b3s")
    nc.sync.dma_start(out=b3s_sb[:], in_=b3s_d[:])

    iota_i = cpool.tile([128, 8], I32, name="iotai", tag="iotai")
    nc.gpsimd.iota(iota_i[:], pattern=[[1, 8]], base=0, channel_multiplier=0)
    iotaneg = cpool.tile([128, 8], F32, name="iotan", tag="iotan")
    nc.vector.tensor_copy(iotaneg[:], iota_i[:])
    nc.vector.tensor_scalar(iotaneg[:], iotaneg[:], -1.0, None, op0=OP.mult)
    negbig = cpool.tile([128, NBLK], F32, name="negbig", tag="negbig")
    nc.vector.memset(negbig[:], BIGNEG)

    # ---- joint threshold search + mask for both images ----
    # Columns of the [128, 4] stat tiles are (img, rt) chains: col = 2*img+rt.
    # Bisection runs in per-row normalized units: values scaled by
    # 1/step0 (step0 = sig*(BR_HI-BR_LO)/4) and centered, so the bracket is
    # the constant [-6, -2] (for BR_LO=-1.05, BR_HI=-0.35) and the per-
    # iteration update is mid += (1-2d)*2^-t -- just 3 small ops per
    # iteration for all 4 chains together.
    def _joint_search(impps, maskbs):
        def st4(tag):
            return stpool.tile([128, 4], F32, name=tag, tag=tag)
        mu4, var4, sig4, step0, inv0 = (st4(t) for t in
                                        ["mu4", "var4", "sig4", "step0", "inv0"])
        nmsc, mid4, cnt4, e4, lofn = (st4(t) for t in
                                      ["nmsc", "mid4", "cnt4", "e4", "lofn"])
        lof4, cntl4, jneg4, thrn4, thr4 = (st4(t) for t in
                                           ["lof4", "cntl4", "jneg4", "thrn4",
                                            "thr4"])
        d4 = stpool.tile([128, 4], U8, name="d4", tag="d4")
        mneg4 = stpool.tile([128, 4], U8, name="mneg4", tag="mneg4")
        agg4 = stpool.tile([128, 4, 2], F32, name="agg4", tag="agg4")

        for col in range(4):
            img, rt = divmod(col, 2)
            bs6 = stpool.tile([128, 2, 6], F32, name=f"bs{col}", tag=f"bs{col}")
            for g in range(2):
                nc.vector.bn_stats(bs6[:, g, :],
                                   impps[img][:, rt, g * 512:(g + 1) * 512])
            nc.vector.bn_aggr(agg4[:, col, :], bs6[:])
        nc.vector.tensor_copy(mu4[:], agg4[:, :, 0])
        nc.vector.tensor_copy(var4[:], agg4[:, :, 1])
        nc.scalar.activation(sig4[:], var4[:], AF.Sqrt)
        nc.vector.tensor_scalar(step0[:], sig4[:], (BR_HI - BR_LO) / 4.0,
                                None, op0=OP.mult)
        nc.vector.reciprocal(inv0[:], step0[:])
        nc.vector.tensor_mul(nmsc[:], mu4[:], inv0[:])
        nc.vector.tensor_scalar(nmsc[:], nmsc[:], -1.0, None, op0=OP.mult)

        # normalized centered bf16 copies (one ACT op per chain)
        impbs = []
        for img in range(2):
            impb = bpool.tile([128, 2, NBLK], BF16, name=f"impb{img}",
                              tag=f"impb{img}")
            impbs.append(impb)
            for rt in range(2):
                col = 2 * img + rt
                nc.scalar.activation(impb[:, rt, :], impps[img][:, rt, :],
                                     AF.Identity, bias=nmsc[:, col:col + 1],
                                     scale=inv0[:, col:col + 1])

        cmid = (BR_LO + BR_HI) / 2.0 / ((BR_HI - BR_LO) / 4.0)
        nc.vector.memset(mid4[:], cmid)
        tbi = 0 if DBG_SKIPSEARCH else DBG_TBI
        for t in range(tbi):
            for col in range(4):
                img, rt = divmod(col, 2)
                scr = scrpool.tile([128, NBLK], BF16, name="scr",
                                   tag=f"scr{col}")
                nc.vector.tensor_scalar(scr[:], impbs[img][:, rt, :],
                                        mid4[:, col:col + 1], None,
                                        op0=OP.is_le, op1=OP.add,
                                        accum_out=cnt4[:, col:col + 1])
            nc.vector.tensor_scalar(d4[:], cnt4[:], float(K_SEARCH), None,
                                    op0=OP.is_ge)
            nc.vector.tensor_scalar(e4[:], d4[:], -(2.0 ** (1 - t)),
                                    2.0 ** (-t), op0=OP.mult, op1=OP.add)
            nc.vector.tensor_add(mid4[:], mid4[:], e4[:])

        # absolute bracket low edge: lof = mu + (mid - 2^(1-T)) * step0
        nc.vector.tensor_scalar(lofn[:], mid4[:], -(2.0 ** (1 - max(tbi, 1))),
                                None, op0=OP.add)
        nc.vector.tensor_mul(lof4[:], lofn[:], step0[:])
        nc.vector.tensor_add(lof4[:], lof4[:], mu4[:])

        # exact f32 finisher, stage-interleaved across the 4 chains
        negimps = []
        for img in range(2):
            negimp = npool.tile([128, 2, NBLK], F32, name=f"negimp{img}",
                                tag=f"negimp{img}")
            negimps.append(negimp)
            nc.scalar.activation(negimp[:], impps[img][:], AF.Copy, scale=-1.0)
        if not DBG_SKIPFIN:
            mles = []
            for col in range(4):
                img, rt = divmod(col, 2)
                mle = scrpool.tile([128, NBLK], U8, name="mle",
                                   tag=f"mleu{col}")
                mles.append(mle)
                nc.vector.tensor_scalar(mle[:], impps[img][:, rt, :],
                                        lof4[:, col:col + 1], None,
                                        op0=OP.is_le, op1=OP.add,
                                        accum_out=cntl4[:, col:col + 1])
            for col in range(4):
                img, rt = divmod(col, 2)
                nc.vector.copy_predicated(negimps[img][:, rt, :], mles[col][:],
                                          negbig[:])
            top8s = []
            for col in range(4):
                img, rt = divmod(col, 2)
                top8 = stpool.tile([128, 8], F32, name=f"top8_{col}",
                                   tag=f"top8_{col}")
                top8s.append(top8)
                nc.vector.max(out=top8[:], in_=negimps[img][:, rt, :])
            nc.vector.tensor_scalar(jneg4[:], cntl4[:], -255.0, None,
                                    op0=OP.add)
            for col in range(4):
                eq8 = stpool.tile([128, 8], F32, name=f"eq8_{col}",
                                  tag=f"eq8_{col}")
                nc.vector.tensor_scalar(eq8[:], iotaneg[:],
                                        jneg4[:, col:col + 1], None,
                                        op0=OP.is_equal)
                nc.vector.tensor_mul(eq8[:], top8s[col][:], eq8[:])
                nc.vector.tensor_reduce(thrn4[:, col:col + 1], eq8[:],
                                        axis=mybir.AxisListType.X, op=OP.add)
            nc.vector.tensor_scalar(thr4[:], thrn4[:], -1.0, None,
                                    op0=OP.mult)
            # cnt_le(lof) >= 256: the bracket edge itself is the threshold
            nc.vector.tensor_scalar(mneg4[:], cntl4[:], 256.0, None,
                                    op0=OP.is_ge)
            nc.vector.copy_predicated(thr4[:], mneg4[:], lof4[:])
        else:
            nc.vector.memset(thr4[:], 0.0)

        # fp8 block mask: Sign(thr - impp) in {-1,0,+1}; host keeps >= 0
        for col in range(4):
            img, rt = divmod(col, 2)
            nc.scalar.activation(maskbs[img][:, rt, :], impps[img][:, rt, :],
                                 AF.Sign, bias=thr4[:, col:col + 1],
                                 scale=-1.0)

    # ---- per-image conv stack ----
    impp_tiles, maskb_tiles = [], []
    for img in range(IMGS):
        x0 = xpool.tile([128, HW], F32, name="x0", tag="x0")
        x1 = xpool.tile([128, HW], F32, name="x1", tag="x1")
        for dc in range(4):
            cs = dc * (HW // 4)
            nc.sync.dma_start(out=x0[:, cs:cs + HW // 4],
                              in_=feats_d[img, 0:128, cs:cs + HW // 4])
            nc.sync.dma_start(out=x1[:, cs:cs + HW // 4],
                              in_=feats_d[img, 128:256, cs:cs + HW // 4])

        # h1 with one-pixel zero halo, [66 x 66]. Partitions 0-63 hold h1;
        # partitions 64-127 hold a copy shifted one column left
        # (u[r, c] = h1pad[r, c+1]) so a single full-height AP feeds the
        # (dy,0)+(dy,1) tap pair of conv2 as one K=128 matmul.
        h1t = h1pool.tile([128, 66 * 66], F32, name="h1", tag="h1")
        h1v = h1t[0:64, :].rearrange("p (r c) -> p r c", c=66)
        h1u = h1t[64:128, :].rearrange("p (r c) -> p r c", c=66)
        nc.vector.memset(h1v[:, 0:1, :], 0.0)
        nc.vector.memset(h1v[:, 65:66, :], 0.0)
        nc.vector.memset(h1v[:, 1:65, 0:1], 0.0)
        nc.vector.memset(h1v[:, 1:65, 65:66], 0.0)
        nc.vector.memset(h1u[:, 0:1, :], 0.0)
        nc.vector.memset(h1u[:, 65:66, :], 0.0)
        nc.vector.memset(h1u[:, 1:65, 64:65], 0.0)

        # conv1 (1x1, M duplicated) + gelu into both h1 copies.
        # Chunk pairs interleaved so the K-accumulation latency of one psum
        # chain hides under the other.
        for cp2 in range(4):
            pts = []
            for half in range(2):
                ci = 2 * cp2 + half
                pts.append(psp.tile([128, 512], F32, name="ps1", tag="ps"))
            for k in range(2):
                for half in range(2):
                    ci = 2 * cp2 + half
                    cs = ci * 512
                    xk = x0 if k == 0 else x1
                    nc.tensor.matmul(pts[half][:], w1_sb[:, k, :],
                                     xk[:, cs:cs + 512],
                                     start=(k == 0), stop=(k == 1))
            for half in range(2):
                ci = 2 * cp2 + half
                pt = pts[half]
                nc.scalar.activation(h1v[:, 1 + 8 * ci:9 + 8 * ci, 1:65],
                                     pt[0:64, :].rearrange("p (r c) -> p r c", c=64),
                                     AF.Gelu, bias=b1_sb[0:64, 0:1])
                nc.scalar.activation(h1u[:, 1 + 8 * ci:9 + 8 * ci, 0:64],
                                     pt[64:128, :].rearrange("p (r c) -> p r c", c=64),
                                     AF.Gelu, bias=b1_sb[64:128, 0:1])

        # conv2 (grouped 3x3 SAME) + gelu -> h2 in 16-row tiles.
        # 4 accumulation chains interleaved (2 c2-tiles x 2 halves) so the
        # psum-accumulate latency overlaps across chains.
        h1f = h1t[:].rearrange("p (r c) -> p r c", c=66)
        h2_tiles = []
        for c2q in range(2):
            quad = []           # (pt, h2t, half, ci)
            for dc in range(2):
                c2 = 2 * c2q + dc
                h2t = h2pool.tile([MID, 1024], F32, name="h2", tag="h2")
                h2_tiles.append(h2t)
                for half in range(2):
                    ci = 2 * c2 + half
                    pt = psp.tile([128, 512], F32, name="ps2", tag="ps")
                    quad.append((pt, h2t, half, ci))
            for t in range(6):
                for pt, h2t, half, ci in quad:
                    r0 = 8 * ci
                    if t < 3:      # pair (dy,0)+(dy,1), K=128 over both copies
                        dy = t
                        lhsT = w2p_sb[:, dy, :]
                        rhs = h1f[:, r0 + dy:r0 + dy + 8, 0:64]
                    else:          # single (dy,2), K=64 lower copy
                        dy = t - 3
                        lhsT = w2s_sb[:, dy, :]
                        rhs = h1v[:, r0 + dy:r0 + dy + 8, 2:66]
                    nc.tensor.matmul(
                        pt[0:64, :].rearrange("p (r c) -> p r c", c=64),
                        lhsT, rhs, start=(t == 0), stop=(t == 5))
            for pt, h2t, half, ci in quad:
                nc.scalar.activation(h2t[:, half * 512:(half + 1) * 512],
                                     pt[0:64, :], AF.Gelu, bias=b2_sb[:, 0:1])

        # conv3 (1x1, 64->256) + bias -> full-res scores tile, one DMA
        sfull = spool.tile([128, 2, HW], F32, name="sfull", tag="sfull")
        if DBG_NOCONV3:
            nc.vector.memset(sfull[:], 0.0)
        for ci in range(8 if not DBG_NOCONV3 else 0):
            h2t = h2_tiles[ci // 2]
            for mt in range(2):
                pt = psp.tile([128, 512], F32, name="ps3", tag="ps")
                nc.tensor.matmul(
                    pt[:], w3_sb[:, mt * 128:(mt + 1) * 128],
                    h2t[:, (ci % 2) * 512:(ci % 2 + 1) * 512],
                    start=True, stop=True)
                if mt == 0:
                    nc.scalar.activation(
                        sfull[:, mt, ci * 512:(ci + 1) * 512], pt[:],
                        AF.Identity, bias=b3s_sb[:, mt:mt + 1])
                else:
                    nc.vector.tensor_scalar(
                        sfull[:, mt, ci * 512:(ci + 1) * 512], pt[:],
                        b3s_sb[:, mt:mt + 1], None, op0=OP.add)
        if not DBG_NOSCORES:
            nc.sync.dma_start(
                out=scores_d[img].rearrange("(t c) w -> c t w", t=2),
                in_=sfull[:])

        # impp = 2x2 block sums of sfull (two big DVE adds)
        impp = ipool.tile([128, 2, NBLK], F32, name="impp", tag="impp")
        if DBG_NOIMPP:
            nc.vector.memset(impp[:], 0.0)
        else:
            sv = sfull[:].rearrange("p t (r c w) -> p t r c w", c=32, w=2)
            cp = cppool.tile([128, 2, 64, 32], F32, name="cp", tag="cp")
            nc.vector.tensor_tensor(cp[:], sv[:, :, :, :, 0], sv[:, :, :, :, 1],
                                    op=OP.add)
            cpv = cp[:].rearrange("p t (r w) c -> p t r w c", w=2)
            nc.vector.tensor_tensor(
                impp[:].rearrange("p t (r c) -> p t r c", c=32),
                cpv[:, :, :, 0, :], cpv[:, :, :, 1, :], op=OP.add)

        maskb = mpool.tile([128, 2, NBLK], FP8, name="maskb", tag="maskb")
        impp_tiles.append(impp)
        maskb_tiles.append(maskb)

    _joint_search(impp_tiles, maskb_tiles)
    if not DBG_NOMASK:
        for img in range(IMGS):
            nc.sync.dma_start(
                out=mask_d[img].rearrange("(t c) w -> c t w", t=2),
                in_=maskb_tiles[img][:])

    for _p in (psp, stpool, mpool, scrpool, npool, bpool, ipool,
               cppool, spool, h2pool, h1pool, xpool, cpool):
        _p.release()


def _prep_weights(w1, b1, w2, b2, w3, b3):
    w1t = np.ascontiguousarray(
        w1[:, :, 0, 0].T.reshape(2, 128, MID).transpose(1, 0, 2)).astype(np.float32)
    w1d = np.concatenate([w1t, w1t], axis=2)      # [128, 2, 128]: M duplicated
    w2t = np.zeros((MID, 9, MID), np.float32)
    for m in range(MID):
        g = m // 8
        for dy in range(3):
            for dx in range(3):
                w2t[g * 8:(g + 1) * 8, 3 * dy + dx, m] = w2[m, :, dy, dx]
    # K=128 pairs: rows 0-63 tap (dy,0), rows 64-127 tap (dy,1); the upper
    # h1 copy is column-shifted -1 so one AP offset feeds both taps.
    w2p = np.stack([np.concatenate([w2t[:, 3 * dy + 0, :],
                                    w2t[:, 3 * dy + 1, :]], axis=0)
                    for dy in range(3)], axis=1)  # [128, 3, 64]
    w2s = np.ascontiguousarray(w2t[:, [2, 5, 8], :])  # [64, 3, 64] taps (dy,2)
    w3t = np.ascontiguousarray(w3[:, :, 0, 0].T).astype(np.float32)
    b3s = np.ascontiguousarray(b3.reshape(2, 128).T).astype(np.float32)
    b1d = np.concatenate([b1.reshape(MID, 1)] * 2, 0).astype(np.float32)
    return dict(w1t=w1d, w2p=w2p, w2s=w2s, w3t=w3t,
                b1=b1d,
                b2=b2.reshape(MID, 1).astype(np.float32),
                b3s=b3s)


_nc_cache = None


def kernel(features, w1, b1, w2, b2, w3, b3, enabled):
    global _nc_cache, LAST_RESULTS
    features = np.asarray(features, np.float32)
    if not int(np.asarray(enabled)):
        return (np.ones((B, C, H, W), np.float32),
                np.zeros((B, C, H, W), np.float32))
    if _nc_cache is None:
        _nc_cache = build_nc()
    nc = _nc_cache
    wmap = _prep_weights(np.asarray(w1), np.asarray(b1), np.asarray(w2),
                         np.asarray(b2), np.asarray(w3), np.asarray(b3))
    fr = features.reshape(B, C, HW)
    in_maps = [dict(features=fr[c * IMGS:(c + 1) * IMGS], **wmap)
               for c in range(N_CORES)]
    res = bass_utils.run_bass_kernel_spmd(nc, in_maps, list(range(N_CORES)),
                                          trace=TRACE)
    LAST_RESULTS = res
    maskb = np.concatenate(
        [np.asarray(res.results[c]["mask"]).astype(np.float32)
         for c in range(N_CORES)], 0)          # [B, C, 1024] in {-1, 0, +1}
    scores = np.concatenate([res.results[c]["scores"] for c in range(N_CORES)], 0)
    blocks = (maskb >= 0).reshape(B, C, 32, 32)
    full = np.broadcast_to(blocks[:, :, :, None, :, None],
                           (B, C, 32, 2, 32, 2)).reshape(B, C, H, W)
    return (full.astype(np.float32),
            scores.reshape(B, C, H, W).astype(np.float32))


if __name__ == "__main__":
    nc = build_nc()
    print("build + compile OK")


# revision 21
# speedup vs baseline: 1.3111x; 1.0033x over previous
"""Trainium2 Bass kernel for nn_EntropyMaskGate.

Pipeline per core (2 images, batch-sharded over 8 cores):
  conv1 (1x1, 256->64) -> gelu -> grouped 3x3 conv (SAME, 8 groups) -> gelu
  -> conv3 (1x1, 64->256) + bias  = entropy_scores (f32, DMA'd full-res)
  2x2 block sums of scores (two big DVE adds) -> per-(b,c)-row 256th-smallest
  threshold (bf16 bisection in sigma-normalized bracket + exact f32 top-8
  finisher) -> Sign-mask written as fp8 block mask; host upsamples 2x2.

Forward-pass note: the reference's STE expression (sg(hard) - sg(soft) + soft)
is exactly `hard` in fp32 round-to-nearest (soft is clipped to [0,1]), so the
mask output is the upsampled hard mask; the soft path is numerically dead.

Precision notes (measured on this hardware):
  - PE fp32 matmul is fp32-exact (rel l2 ~1e-7 vs fp64); fp32r is tf32-like
    (1.4e-4) so it is NOT used: impp = pool(conv3) feeds the mask and needs
    exact ordering.
  - Bisection counts run on bf16 copies (centered per row); only the bracket
    comes from them, the threshold itself is recovered from exact f32 values
    via top-8 (max8) with a 0.005*sigma guard band on the bracket low edge.
  - ACT Sign(thr - impp) -> {-1,0,+1} exact in fp8_e4m3; host keeps >= 0
    (sign==0 is the threshold element itself, which reference keeps).
"""

import numpy as np

import concourse.bass as bass
import concourse.mybir as mybir
from concourse import bacc, bass_utils
from concourse.tile import TileContext

F32 = mybir.dt.float32
BF16 = mybir.dt.bfloat16
FP8 = mybir.dt.float8e4
I32 = mybir.dt.int32
U8 = mybir.dt.uint8
AF = mybir.ActivationFunctionType
OP = mybir.AluOpType

B, C, H, W = 16, 256, 64, 64
MID, GROUPS = 64, 8
N_CORES = 8
IMGS = B // N_CORES          # 2 images per core
HW = H * W                   # 4096
NBLK = 1024                  # 32*32 blocks per (b,c) row
KEEP = 256                   # blocks kept per row
T_BISECT = 12                # bisection iterations
BR_LO = -1.05                # bracket in centered/sigma units around the
BR_HI = -0.35                # 25th-percentile threshold (z ~ -0.674)
K_SEARCH = 253               # bf16 bisection target: 3 ranks below KEEP
                             # absorbs bf16-vs-f32 count slack (+-~2)
                             # while keeping the finisher rank in [1,8]
BIGNEG = -(2.0 ** 96)

TRACE = False
LAST_RESULTS = None
import os as _os
DBG_TBI = int(_os.environ.get("KM_TBI", str(T_BISECT)))
DBG_SKIPFIN = int(_os.environ.get("KM_SKIPFIN", "0"))
DBG_SKIPSEARCH = int(_os.environ.get("KM_SKIPSEARCH", "0"))
DBG_NOMASK = int(_os.environ.get("KM_NOMASK", "0"))
DBG_NOSCORES = int(_os.environ.get("KM_NOSCORES", "0"))
DBG_NOIMPP = int(_os.environ.get("KM_NOIMPP", "0"))
DBG_NOCONV3 = int(_os.environ.get("KM_NOCONV3", "0"))


def build_nc(repeat=1):
    nc = bacc.Bacc("TRN2", target_bir_lowering=False, debug=False,
                   num_devices=N_CORES)

    feats_d = nc.dram_tensor("features", [IMGS, C, HW], F32, kind="ExternalInput").ap()
    w1t_d = nc.dram_tensor("w1t", [128, 2, 128], F32, kind="ExternalInput").ap()
    w2p_d = nc.dram_tensor("w2p", [128, 3, MID], F32, kind="ExternalInput").ap()
    w2s_d = nc.dram_tensor("w2s", [MID, 3, MID], F32, kind="ExternalInput").ap()
    w3t_d = nc.dram_tensor("w3t", [MID, C], F32, kind="ExternalInput").ap()
    b1_d = nc.dram_tensor("b1", [128, 1], F32, kind="ExternalInput").ap()
    b2_d = nc.dram_tensor("b2", [MID, 1], F32, kind="ExternalInput").ap()
    b3s_d = nc.dram_tensor("b3s", [128, 2], F32, kind="ExternalInput").ap()
    scores_d = nc.dram_tensor("scores", [IMGS, C, HW], F32, kind="ExternalOutput").ap()
    mask_d = nc.dram_tensor("mask", [IMGS, C, NBLK], U8, kind="ExternalOutput").ap()

    with TileContext(nc) as tc:
        for _rep in range(repeat):
            _build(nc, tc, feats_d, w1t_d, w2p_d, w2s_d, w3t_d, b1_d, b2_d,
                   b3s_d, scores_d, mask_d)
    nc.compile()
    return nc


def _build(nc, tc, feats_d, w1t_d, w2p_d, w2s_d, w3t_d, b1_d, b2_d, b3s_d,
           scores_d, mask_d):
    cpool = tc.alloc_tile_pool(name="consts", bufs=1)
    xpool = tc.alloc_tile_pool(name="x", bufs=1)
    h1pool = tc.alloc_tile_pool(name="h1", bufs=2)
    h2pool = tc.alloc_tile_pool(name="h2", bufs=6)
    spool = tc.alloc_tile_pool(name="s", bufs=1)
    cppool = tc.alloc_tile_pool(name="cp", bufs=1)
    ipool = tc.alloc_tile_pool(name="impp", bufs=2)
    bpool = tc.alloc_tile_pool(name="impb", bufs=1)  # 2 tags, one per img
    npool = tc.alloc_tile_pool(name="neg", bufs=1)  # 2 tags, one per img
    scrpool = tc.alloc_tile_pool(name="scr", bufs=1)
    mpool = tc.alloc_tile_pool(name="maskb", bufs=2)
    stpool = tc.alloc_tile_pool(name="stats", bufs=1)
    psp = tc.alloc_tile_pool(name="psp", bufs=8, space="PSUM")

    # ---- constants ----
    w1_sb = cpool.tile([128, 2, 128], F32, name="w1", tag="w1")
    nc.sync.dma_start(out=w1_sb[:], in_=w1t_d[:])
    w2p_sb = cpool.tile([128, 3, MID], F32, name="w2p", tag="w2p")
    nc.sync.dma_start(out=w2p_sb[:], in_=w2p_d[:])
    w2s_sb = cpool.tile([MID, 3, MID], F32, name="w2s", tag="w2s")
    nc.sync.dma_start(out=w2s_sb[:], in_=w2s_d[:])
    w3_sb = cpool.tile([MID, C], F32, name="w3", tag="w3")
    nc.sync.dma_start(out=w3_sb[:], in_=w3t_d[:])
    b1_sb = cpool.tile([128, 1], F32, name="b1", tag="b1")
    nc.sync.dma_start(out=b1_sb[:], in_=b1_d[:])
    b2_sb = cpool.tile([MID, 1], F32, name="b2", tag="b2")
    nc.sync.dma_start(out=b2_sb[:], in_=b2_d[:])
    b3s_sb = cpool.tile([128, 2], F32, name="b3s", tag="b3s")
    nc.sync.dma_start(out=b3s_sb[:], in_=b3s_d[:])

    iota_i = cpool.tile([128, 8], I32, name="iotai", tag="iotai")
    nc.gpsimd.iota(iota_i[:], pattern=[[1, 8]], base=0, channel_multiplier=0)
    iotaneg = cpool.tile([128, 8], F32, name="iotan", tag="iotan")
    nc.vector.tensor_copy(iotaneg[:], iota_i[:])
    nc.vector.tensor_scalar(iotaneg[:], iotaneg[:], -1.0, None, op0=OP.mult)
    negbig = cpool.tile([128, NBLK], F32, name="negbig", tag="negbig")
    nc.vector.memset(negbig[:], BIGNEG)

    # ---- joint threshold search + mask for both images ----
    # Columns of the [128, 4] stat tiles are (img, rt) chains: col = 2*img+rt.
    # Bisection runs in per-row normalized units: values scaled by
    # 1/step0 (step0 = sig*(BR_HI-BR_LO)/4) and centered, so the bracket is
    # the constant [-6, -2] (for BR_LO=-1.05, BR_HI=-0.35) and the per-
    # iteration update is mid += (1-2d)*2^-t -- just 3 small ops per
    # iteration for all 4 chains together.
    def _joint_search(impps, maskbs):
        def st4(tag):
            return stpool.tile([128, 4], F32, name=tag, tag=tag)
        mu4, var4, sig4, step0, inv0 = (st4(t) for t in
                                        ["mu4", "var4", "sig4", "step0", "inv0"])
        nmsc, mid4, cnt4, e4, lofn = (st4(t) for t in
                                      ["nmsc", "mid4", "cnt4", "e4", "lofn"])
        lof4, cntl4, jneg4, thrn4, thr4 = (st4(t) for t in
                                           ["lof4", "cntl4", "jneg4", "thrn4",
                                            "thr4"])
        d4 = stpool.tile([128, 4], U8, name="d4", tag="d4")
        mneg4 = stpool.tile([128, 4], U8, name="mneg4", tag="mneg4")
        agg4 = stpool.tile([128, 4, 2], F32, name="agg4", tag="agg4")

        for col in range(4):
            img, rt = divmod(col, 2)
            bs6 = stpool.tile([128, 2, 6], F32, name=f"bs{col}", tag=f"bs{col}")
            for g in range(2):
                nc.vector.bn_stats(bs6[:, g, :],
                                   impps[img][:, rt, g * 512:(g + 1) * 512])
            nc.vector.bn_aggr(agg4[:, col, :], bs6[:])
        nc.vector.tensor_copy(mu4[:], agg4[:, :, 0])
        nc.vector.tensor_copy(var4[:], agg4[:, :, 1])
        nc.scalar.activation(sig4[:], var4[:], AF.Sqrt)
        nc.vector.tensor_scalar(step0[:], sig4[:], (BR_HI - BR_LO) / 4.0,
                                None, op0=OP.mult)
        nc.vector.reciprocal(inv0[:], step0[:])
        nc.vector.tensor_mul(nmsc[:], mu4[:], inv0[:])
        nc.vector.tensor_scalar(nmsc[:], nmsc[:], -1.0, None, op0=OP.mult)

        # normalized centered bf16 copies (DVE: keeps the whole search tail
        # on one engine so the in-order ACT queue never blocks on it)
        impbs = []
        for img in range(2):
            impb = bpool.tile([128, 2, NBLK], BF16, name=f"impb{img}",
                              tag=f"impb{img}")
            impbs.append(impb)
            for rt in range(2):
                col = 2 * img + rt
                nc.vector.tensor_scalar(impb[:, rt, :], impps[img][:, rt, :],
                                        inv0[:, col:col + 1],
                                        nmsc[:, col:col + 1],
                                        op0=OP.mult, op1=OP.add)
        # negated f32 copies for the finisher (independent of the bisection;
        # emitted early so the DVE queue stays dense)
        negimps = []
        for img in range(2):
            negimp = npool.tile([128, 2, NBLK], F32, name=f"negimp{img}",
                                tag=f"negimp{img}")
            negimps.append(negimp)
            nc.vector.tensor_scalar(negimp[:], impps[img][:], -1.0, None,
                                    op0=OP.mult)

        cmid = (BR_LO + BR_HI) / 2.0 / ((BR_HI - BR_LO) / 4.0)
        nc.vector.memset(mid4[:], cmid)
        tbi = 0 if DBG_SKIPSEARCH else DBG_TBI
        for t in range(tbi):
            for col in range(4):
                img, rt = divmod(col, 2)
                scr = scrpool.tile([128, NBLK], BF16, name="scr",
                                   tag=f"scr{col}")
                nc.vector.tensor_scalar(scr[:], impbs[img][:, rt, :],
                                        mid4[:, col:col + 1], None,
                                        op0=OP.is_le, op1=OP.add,
                                        accum_out=cnt4[:, col:col + 1])
            nc.vector.tensor_scalar(d4[:], cnt4[:], float(K_SEARCH), None,
                                    op0=OP.is_ge)
            nc.vector.tensor_scalar(e4[:], d4[:], -(2.0 ** (1 - t)),
                                    2.0 ** (-t), op0=OP.mult, op1=OP.add)
            nc.vector.tensor_add(mid4[:], mid4[:], e4[:])

        # absolute bracket low edge: lof = mu + (mid - 2^(1-T)) * step0
        nc.vector.tensor_scalar(lofn[:], mid4[:], -(2.0 ** (1 - max(tbi, 1))),
                                None, op0=OP.add)
        nc.vector.tensor_mul(lof4[:], lofn[:], step0[:])
        nc.vector.tensor_add(lof4[:], lof4[:], mu4[:])

        # exact f32 finisher, stage-interleaved across the 4 chains
        if not DBG_SKIPFIN:
            mles = []
            for col in range(4):
                img, rt = divmod(col, 2)
                mle = scrpool.tile([128, NBLK], U8, name="mle",
                                   tag=f"mleu{col}")
                mles.append(mle)
                nc.vector.tensor_scalar(mle[:], impps[img][:, rt, :],
                                        lof4[:, col:col + 1], None,
                                        op0=OP.is_le, op1=OP.add,
                                        accum_out=cntl4[:, col:col + 1])
            for col in range(4):
                img, rt = divmod(col, 2)
                nc.vector.copy_predicated(negimps[img][:, rt, :], mles[col][:],
                                          negbig[:])
            top8s = []
            for col in range(4):
                img, rt = divmod(col, 2)
                top8 = stpool.tile([128, 8], F32, name=f"top8_{col}",
                                   tag=f"top8_{col}")
                top8s.append(top8)
                nc.vector.max(out=top8[:], in_=negimps[img][:, rt, :])
            nc.vector.tensor_scalar(jneg4[:], cntl4[:], -255.0, None,
                                    op0=OP.add)
            for col in range(4):
                eq8 = stpool.tile([128, 8], F32, name=f"eq8_{col}",
                                  tag=f"eq8_{col}")
                nc.vector.tensor_scalar(eq8[:], iotaneg[:],
                                        jneg4[:, col:col + 1], None,
                                        op0=OP.is_equal)
                nc.vector.tensor_mul(eq8[:], top8s[col][:], eq8[:])
                nc.vector.tensor_reduce(thrn4[:, col:col + 1], eq8[:],
                                        axis=mybir.AxisListType.X, op=OP.add)
            nc.vector.tensor_scalar(thr4[:], thrn4[:], -1.0, None,
                                    op0=OP.mult)
            # cnt_le(lof) >= 256: the bracket edge itself is the threshold
            nc.vector.tensor_scalar(mneg4[:], cntl4[:], 256.0, None,
                                    op0=OP.is_ge)
            nc.vector.copy_predicated(thr4[:], mneg4[:], lof4[:])
        else:
            nc.vector.memset(thr4[:], 0.0)

        # u8 block mask: impp <= thr (DVE; host upsamples 2x2)
        for col in range(4):
            img, rt = divmod(col, 2)
            nc.vector.tensor_scalar(maskbs[img][:, rt, :],
                                    impps[img][:, rt, :],
                                    thr4[:, col:col + 1], None, op0=OP.is_le)

    # ---- per-image conv stack ----
    impp_tiles, maskb_tiles = [], []
    for img in range(IMGS):
        x0 = xpool.tile([128, HW], F32, name="x0", tag="x0")
        x1 = xpool.tile([128, HW], F32, name="x1", tag="x1")
        for dc in range(4):
            cs = dc * (HW // 4)
            nc.sync.dma_start(out=x0[:, cs:cs + HW // 4],
                              in_=feats_d[img, 0:128, cs:cs + HW // 4])
            nc.sync.dma_start(out=x1[:, cs:cs + HW // 4],
                              in_=feats_d[img, 128:256, cs:cs + HW // 4])

        # h1 with one-pixel zero halo, [66 x 66]. Partitions 0-63 hold h1;
        # partitions 64-127 hold a copy shifted one column left
        # (u[r, c] = h1pad[r, c+1]) so a single full-height AP feeds the
        # (dy,0)+(dy,1) tap pair of conv2 as one K=128 matmul.
        h1t = h1pool.tile([128, 66 * 66], F32, name="h1", tag="h1")
        h1v = h1t[0:64, :].rearrange("p (r c) -> p r c", c=66)
        h1u = h1t[64:128, :].rearrange("p (r c) -> p r c", c=66)
        nc.vector.memset(h1v[:, 0:1, :], 0.0)
        nc.vector.memset(h1v[:, 65:66, :], 0.0)
        nc.vector.memset(h1v[:, 1:65, 0:1], 0.0)
        nc.vector.memset(h1v[:, 1:65, 65:66], 0.0)
        nc.vector.memset(h1u[:, 0:1, :], 0.0)
        nc.vector.memset(h1u[:, 65:66, :], 0.0)
        nc.vector.memset(h1u[:, 1:65, 64:65], 0.0)

        # conv1 (1x1, M duplicated) + gelu into both h1 copies.
        # Chunk pairs interleaved so the K-accumulation latency of one psum
        # chain hides under the other.
        for cp2 in range(4):
            pts = []
            for half in range(2):
                ci = 2 * cp2 + half
                pts.append(psp.tile([128, 512], F32, name="ps1", tag="ps"))
            for k in range(2):
                for half in range(2):
                    ci = 2 * cp2 + half
                    cs = ci * 512
                    xk = x0 if k == 0 else x1
                    nc.tensor.matmul(pts[half][:], w1_sb[:, k, :],
                                     xk[:, cs:cs + 512],
                                     start=(k == 0), stop=(k == 1))
            for half in range(2):
                ci = 2 * cp2 + half
                pt = pts[half]
                nc.scalar.activation(h1v[:, 1 + 8 * ci:9 + 8 * ci, 1:65],
                                     pt[0:64, :].rearrange("p (r c) -> p r c", c=64),
                                     AF.Gelu, bias=b1_sb[0:64, 0:1])
                nc.scalar.activation(h1u[:, 1 + 8 * ci:9 + 8 * ci, 0:64],
                                     pt[64:128, :].rearrange("p (r c) -> p r c", c=64),
                                     AF.Gelu, bias=b1_sb[64:128, 0:1])

        # conv2 (grouped 3x3 SAME) + gelu -> h2 in 16-row tiles.
        # 4 accumulation chains interleaved (2 c2-tiles x 2 halves) so the
        # psum-accumulate latency overlaps across chains.
        h1f = h1t[:].rearrange("p (r c) -> p r c", c=66)
        h2_tiles = []
        for c2q in range(2):
            quad = []           # (pt, h2t, half, ci)
            for dc in range(2):
                c2 = 2 * c2q + dc
                h2t = h2pool.tile([MID, 1024], F32, name="h2", tag="h2")
                h2_tiles.append(h2t)
                for half in range(2):
                    ci = 2 * c2 + half
                    pt = psp.tile([128, 512], F32, name="ps2", tag="ps")
                    quad.append((pt, h2t, half, ci))
            for t in range(6):
                for pt, h2t, half, ci in quad:
                    r0 = 8 * ci
                    if t < 3:      # pair (dy,0)+(dy,1), K=128 over both copies
                        dy = t
                        lhsT = w2p_sb[:, dy, :]
                        rhs = h1f[:, r0 + dy:r0 + dy + 8, 0:64]
                    else:          # single (dy,2), K=64 lower copy
                        dy = t - 3
                        lhsT = w2s_sb[:, dy, :]
                        rhs = h1v[:, r0 + dy:r0 + dy + 8, 2:66]
                    nc.tensor.matmul(
                        pt[0:64, :].rearrange("p (r c) -> p r c", c=64),
                        lhsT, rhs, start=(t == 0), stop=(t == 5))
            for pt, h2t, half, ci in quad:
                nc.scalar.activation(h2t[:, half * 512:(half + 1) * 512],
                                     pt[0:64, :], AF.Gelu, bias=b2_sb[:, 0:1])

        # conv3 (1x1, 64->256) + bias -> full-res scores tile, one DMA
        sfull = spool.tile([128, 2, HW], F32, name="sfull", tag="sfull")
        if DBG_NOCONV3:
            nc.vector.memset(sfull[:], 0.0)
        for ci in range(8 if not DBG_NOCONV3 else 0):
            h2t = h2_tiles[ci // 2]
            for mt in range(2):
                pt = psp.tile([128, 512], F32, name="ps3", tag="ps")
                nc.tensor.matmul(
                    pt[:], w3_sb[:, mt * 128:(mt + 1) * 128],
                    h2t[:, (ci % 2) * 512:(ci % 2 + 1) * 512],
                    start=True, stop=True)
                if mt == 0:
                    nc.scalar.activation(
                        sfull[:, mt, ci * 512:(ci + 1) * 512], pt[:],
                        AF.Identity, bias=b3s_sb[:, mt:mt + 1])
                else:
                    nc.vector.tensor_scalar(
                        sfull[:, mt, ci * 512:(ci + 1) * 512], pt[:],
                        b3s_sb[:, mt:mt + 1], None, op0=OP.add)
        if not DBG_NOSCORES:
            nc.sync.dma_start(
                out=scores_d[img].rearrange("(t c) w -> c t w", t=2),
                in_=sfull[:])

        # impp = 2x2 block sums of sfull (two big DVE adds)
        impp = ipool.tile([128, 2, NBLK], F32, name="impp", tag="impp")
        if DBG_NOIMPP:
            nc.vector.memset(impp[:], 0.0)
        else:
            sv = sfull[:].rearrange("p t (r c w) -> p t r c w", c=32, w=2)
            cp = cppool.tile([128, 2, 64, 32], F32, name="cp", tag="cp")
            nc.vector.tensor_tensor(cp[:], sv[:, :, :, :, 0], sv[:, :, :, :, 1],
                                    op=OP.add)
            cpv = cp[:].rearrange("p t (r w) c -> p t r w c", w=2)
            nc.vector.tensor_tensor(
                impp[:].rearrange("p t (r c) -> p t r c", c=32),
                cpv[:, :, :, 0, :], cpv[:, :, :, 1, :], op=OP.add)

        maskb = mpool.tile([128, 2, NBLK], U8, name="maskb", tag="maskb")
        impp_tiles.append(impp)
        maskb_tiles.append(maskb)

    _joint_search(impp_tiles, maskb_tiles)
    if not DBG_NOMASK:
        for img in range(IMGS):
            nc.sync.dma_start(
                out=mask_d[img].rearrange("(t c) w -> c t w", t=2),
                in_=maskb_tiles[img][:])

    for _p in (psp, stpool, mpool, scrpool, npool, bpool, ipool,
               cppool, spool, h2pool, h1pool, xpool, cpool):
        _p.release()


def _prep_weights(w1, b1, w2, b2, w3, b3):
    w1t = np.ascontiguousarray(
        w1[:, :, 0, 0].T.reshape(2, 128, MID).transpose(1, 0, 2)).astype(np.float32)
    w1d = np.concatenate([w1t, w1t], axis=2)      # [128, 2, 128]: M duplicated
    w2t = np.zeros((MID, 9, MID), np.float32)
    for m in range(MID):
        g = m // 8
        for dy in range(3):
            for dx in range(3):
                w2t[g * 8:(g + 1) * 8, 3 * dy + dx, m] = w2[m, :, dy, dx]
    # K=128 pairs: rows 0-63 tap (dy,0), rows 64-127 tap (dy,1); the upper
    # h1 copy is column-shifted -1 so one AP offset feeds both taps.
    w2p = np.stack([np.concatenate([w2t[:, 3 * dy + 0, :],
                                    w2t[:, 3 * dy + 1, :]], axis=0)
                    for dy in range(3)], axis=1)  # [128, 3, 64]
    w2s = np.ascontiguousarray(w2t[:, [2, 5, 8], :])  # [64, 3, 64] taps (dy,2)
    w3t = np.ascontiguousarray(w3[:, :, 0, 0].T).astype(np.float32)
    b3s = np.ascontiguousarray(b3.reshape(2, 128).T).astype(np.float32)
    b1d = np.concatenate([b1.reshape(MID, 1)] * 2, 0).astype(np.float32)
    return dict(w1t=w1d, w2p=w2p, w2s=w2s, w3t=w3t,
                b1=b1d,
                b2=b2.reshape(MID, 1).astype(np.float32),
                b3s=b3s)


_nc_cache = None


def kernel(features, w1, b1, w2, b2, w3, b3, enabled):
    global _nc_cache, LAST_RESULTS
    features = np.asarray(features, np.float32)
    if not int(np.asarray(enabled)):
        return (np.ones((B, C, H, W), np.float32),
                np.zeros((B, C, H, W), np.float32))
    if _nc_cache is None:
        _nc_cache = build_nc()
    nc = _nc_cache
    wmap = _prep_weights(np.asarray(w1), np.asarray(b1), np.asarray(w2),
                         np.asarray(b2), np.asarray(w3), np.asarray(b3))
    fr = features.reshape(B, C, HW)
    in_maps = [dict(features=fr[c * IMGS:(c + 1) * IMGS], **wmap)
               for c in range(N_CORES)]
    res = bass_utils.run_bass_kernel_spmd(nc, in_maps, list(range(N_CORES)),
                                          trace=TRACE)
    LAST_RESULTS = res
    maskb = np.concatenate(
        [np.asarray(res.results[c]["mask"]) for c in range(N_CORES)], 0)
    scores = np.concatenate([res.results[c]["scores"] for c in range(N_CORES)], 0)
    blocks = (maskb != 0).reshape(B, C, 32, 32)
    full = np.broadcast_to(blocks[:, :, :, None, :, None],
                           (B, C, 32, 2, 32, 2)).reshape(B, C, H, W)
    return (full.astype(np.float32),
            scores.reshape(B, C, H, W).astype(np.float32))


if __name__ == "__main__":
    nc = build_nc()
    print("build + compile OK")
